# revision 22
# baseline (speedup 1.0000x reference)
"""Trainium2 Bass kernel for nn_NodeClassifier (gnn_message_passing).

Strategy (8 NeuronCores, SPMD):
  - Nodes block-partitioned by id across 8 cores (6250 each, padded to 6272).
  - Edges partitioned by dst core, grouped per 128-node dst tile, split into
    two src-token ranges (A: tok < 32768, B: rest) so gather indices fit
    int16, and padded to 128-edge blocks (block structure shared across
    cores = max over cores).
  - Aggregation is a segment-sum done ON THE TENSOR ENGINE: per 128-edge
    block, a [128 edges x 128 nodes] 0/1 selection matrix S (built on the
    vector engine from compact per-edge dst slots via iota==d) multiplies
    the gathered [128 edges x 128 feat] fp16 payload, accumulating into a
    per-tile PSUM [128 nodes x 128 feat]. A leading zero-matmul clears the
    accumulator so has_written semantics are never relied on.
  - Layer-0 payload is host-expanded (contiguous DMA streams). Layer-1
    payload is fetched with batched dma_gather (custom SWDGE instruction,
    1024 indices per call) from the AllGathered raw-v table.
  - All dense compute (GCN linear, BN, FF, cls) in fp16 matmuls,
    feature-major. BN stats via per-chunk DVE reduce + ACT Square accum_out,
    AllReduced (tiny). b_gcn dropped (BN(z+const)==BN(z), exact).
  - Layer-0 BN2 folded across the halo exchange: AllGather ships RAW v;
    a2 folds into layer-1's GCN weight, c2 via a rank-1 (c2^T W) x mask
    matmul and the local residual. Layer-1 BN2 folds into the classifier.
  - Weights replicated. Program identical on all cores.
"""

import os
import sys
import numpy as np

for _p in ("/opt/trn_rl_repo",):
    if _p not in sys.path and os.path.isdir(_p):
        sys.path.insert(0, _p)

from contextlib import ExitStack

import concourse.bass as bass
import concourse.bacc as bacc
import concourse.mybir as mybir
import concourse.tile as tile
from concourse.bass_utils import run_bass_kernel_spmd

F32 = mybir.dt.float32
F16 = mybir.dt.float16
I16 = mybir.dt.int16
AF = mybir.ActivationFunctionType
ALU = mybir.AluOpType

CORES = 8
D = 128
H = 512
DEPTH = 2
EPS = 1e-5
CHUNK = 512
H1 = 4096              # per-core row split: part 1 rows [0,4096)
H2 = 2176              # part 2 rows [4096,6272); 7*4096+4095 = 32767 = int16 max
BLK = 128              # edges per gather/matmul block
GBLK = 8               # blocks per dma_gather (1024-descriptor ring cap)


# ----------------------------------------------------------------------------
# Host-side preparation
# ----------------------------------------------------------------------------

def _prepare(nodes, edge_src, edge_dst):
    N = nodes.shape[0]
    assert N % CORES == 0
    sh_real = N // CORES
    nt = -(-sh_real // 128)
    sh = nt * 128
    if sh == sh_real:
        nt += 1
        sh += 128
    tok_n = CORES * sh

    # permutation: per core block, sort nodes by degree ascending (keeps the
    # dense phase layout of the earlier kernel; not load-bearing here)
    deg = np.bincount(edge_dst, minlength=N).astype(np.int64)
    tok_of_node = np.empty(N, np.int64)
    node_of_tok = np.full(tok_n, -1, np.int64)
    for c in range(CORES):
        ids = np.arange(c * sh_real, (c + 1) * sh_real)
        order = np.argsort(deg[ids], kind="stable")
        toks = c * sh + np.arange(sh_real)
        tok_of_node[ids[order]] = toks
        node_of_tok[toks] = ids[order]

    dst_tok = tok_of_node[edge_dst]
    src_tok = tok_of_node[edge_src]

    e_core = dst_tok // sh
    e_slot = dst_tok % sh
    e_t = e_slot // 128
    e_p = e_slot % 128
    s_core = src_tok // sh
    s_slot = src_tok % sh
    e_r = (s_slot >= H1).astype(np.int64)

    # per (core, tile, range) edge counts -> shared block counts
    cnt = np.zeros((CORES, nt, 2), np.int64)
    np.add.at(cnt, (e_core, e_t, e_r), 1)
    nblk_t = np.maximum(-(-cnt.max(axis=0) // BLK), 1)  # [nt, 2]
    blkoff = np.zeros((nt, 2), np.int64)
    nblkR = [0, 0]
    for r in range(2):
        off = 0
        for t in range(nt):
            blkoff[t, r] = off
            off += nblk_t[t, r]
        nblkR[r] = off

    # per-core edge placement: edge -> (range, global block, lane)
    # order within (core, tile, range): stable original order
    idx_arr = [np.zeros((CORES, nblkR[r] * BLK), np.int64) for r in range(2)]
    dloc = [np.full((CORES, nblkR[r] * BLK), 999.0, np.float32) for r in range(2)]
    paytok = [np.full((CORES, nblkR[r] * BLK), -1, np.int64) for r in range(2)]
    order = np.lexsort((np.arange(len(dst_tok)), e_r, e_t, e_core))
    # rank within (core, tile, range)
    key = ((e_core * nt + e_t) * 2 + e_r)
    ks = key[order]
    starts = np.searchsorted(ks, np.arange(CORES * nt * 2), side="left")
    rank = np.arange(len(order)) - starts[ks]
    ec, et, er, ep = e_core[order], e_t[order], e_r[order], e_p[order]
    st = src_tok[order]
    sc, ss = s_core[order], s_slot[order]
    row_r = [sc * H1 + ss, sc * H2 + (ss - H1)]
    pos = (blkoff[et, er] * BLK + rank)
    for r in range(2):
        m = er == r
        idx_arr[r][ec[m], pos[m]] = row_r[r][m]
        dloc[r][ec[m], pos[m]] = ep[m]
        paytok[r][ec[m], pos[m]] = st[m]

    # invdeg [128, nt] per core (0 for dummy slots)
    cnt_tok = np.bincount(dst_tok, minlength=tok_n)
    deg_tok = cnt_tok.reshape(CORES, sh)
    node_ok = node_of_tok.reshape(CORES, sh) >= 0
    iv = (1.0 / np.maximum(deg_tok, 1.0)) * node_ok
    mask = ((deg_tok > 0) & node_ok).astype(np.float16)
    invdeg = np.zeros((CORES, 128, nt), np.float32)
    for c in range(CORES):
        invdeg[c] = iv[c].reshape(nt, 128).T

    # replicated full node table [tok_n, D]
    table0 = np.zeros((tok_n, D), np.float32)
    real = node_of_tok >= 0
    table0[real] = nodes[node_of_tok[real]]
    t16 = table0.astype(np.float16)

    # layer-0 payload streams (edge-blocked x0 rows), fp16
    pay = []
    for r in range(2):
        p = np.zeros((CORES, nblkR[r] * BLK, D), np.float16)
        valid = paytok[r] >= 0
        p[valid] = t16[paytok[r][valid]]
        # gather layout: index i -> (partition i%128, block i//128)
        p = p.reshape(CORES, nblkR[r], BLK, D).transpose(0, 2, 1, 3)
        pay.append(np.ascontiguousarray(p.reshape(CORES, 128, nblkR[r] * D)))

    # wrapped int16 index arrays [128, nblkR*8] (replicated across 8 Q7 cores)
    idxw = []
    for r in range(2):
        w = idx_arr[r].reshape(CORES, -1, 16)  # [C, nblk*8, 16]
        w = w.transpose(0, 2, 1).astype(np.int16)  # [C, 16, nblk*8]
        idxw.append(np.ascontiguousarray(np.tile(w, (1, 8, 1))))

    # d_rel [128, nsub] fp16: per tile, its A-blocks then B-blocks
    # (lane -> partition)
    nsub_t = nblk_t.sum(axis=1)
    suboff = np.concatenate([[0], np.cumsum(nsub_t)])
    nsub = int(suboff[-1])
    drel = np.zeros((CORES, 128, nsub), np.float16)
    for t in range(nt):
        s0 = suboff[t]
        for r in range(2):
            b0, nb = blkoff[t, r], nblk_t[t, r]
            seg = dloc[r][:, b0 * BLK:(b0 + nb) * BLK].reshape(CORES, nb, BLK)
            drel[:, :, s0:s0 + nb] = seg.transpose(0, 2, 1).astype(np.float16)
            s0 += nb

    maxsub = int(nsub_t.max())
    iota_rep = np.tile(np.arange(128, dtype=np.float16), maxsub)[None, :]
    iota_rep = np.ascontiguousarray(np.broadcast_to(
        iota_rep, (128, maxsub * 128)))

    return dict(
        N=N, sh_real=sh_real, sh=sh, nt=nt, tok_n=tok_n,
        nblk_t=nblk_t, blkoff=blkoff, nblkR=nblkR,
        nsub_t=[int(x) for x in nsub_t], suboff=[int(x) for x in suboff],
        maxsub=maxsub, iota_rep=iota_rep,
        idxw=idxw, pay=pay, drel=drel,
        invdeg=invdeg, mask=mask, table0=table0, node_of_tok=node_of_tok,
    )


# ----------------------------------------------------------------------------
# Program builder
# ----------------------------------------------------------------------------

def build_program(cfg):
    nt, sh, sh_real = cfg["nt"], cfg["sh"], cfg["sh_real"]
    tok_n = cfg["tok_n"]
    nblk_t, blkoff, nblkR = cfg["nblk_t"], cfg["blkoff"], cfg["nblkR"]
    nsub_t, suboff, maxsub = cfg["nsub_t"], cfg["suboff"], cfg["maxsub"]
    N = cfg["N"]
    rg = [list(range(CORES))]

    chunks = []
    c0 = 0
    while c0 < sh:
        cw = min(CHUNK, sh - c0)
        chunks.append((c0, cw))
        c0 += cw
    nch = len(chunks)

    nc = bacc.Bacc("TRN2", target_bir_lowering=False, debug=False,
                   num_devices=CORES, num_swdge_queues=4)

    # ---- I/O declarations
    pay_d = [nc.dram_tensor(f"pay{r}", [128, nblkR[r] * D], F16,
                            kind="ExternalInput") for r in range(2)]
    idx_d = [nc.dram_tensor(f"idx{r}", [128, nblkR[r] * 8], I16,
                            kind="ExternalInput") for r in range(2)]
    drel_d = nc.dram_tensor("drel", [128, suboff[-1]], F16,
                            kind="ExternalInput")
    iota_d = nc.dram_tensor("iotar", [128, maxsub * 128], F16,
                            kind="ExternalInput")
    x0_d = nc.dram_tensor("x016", [D, sh], F16, kind="ExternalInput")
    invdeg_d = nc.dram_tensor("invdeg", [128, nt], F32, kind="ExternalInput")
    mask_d = nc.dram_tensor("mask16", [1, sh], F16, kind="ExternalInput")
    wg_d = [nc.dram_tensor(f"wg{l}", [D, D], F16, kind="ExternalInput")
            for l in range(DEPTH)]
    w1_d = [nc.dram_tensor(f"w1_{l}", [D, H], F16, kind="ExternalInput")
            for l in range(DEPTH)]
    fb1_d = [nc.dram_tensor(f"fb1_{l}", [D, H // D], F32, kind="ExternalInput")
             for l in range(DEPTH)]
    w2_d = [nc.dram_tensor(f"w2_{l}", [H, D], F16, kind="ExternalInput")
            for l in range(DEPTH)]
    bn_d = {}
    for l in range(DEPTH):
        for nm in ("g1", "b1", "g2", "b2"):
            bn_d[(nm, l)] = nc.dram_tensor(f"{nm}_{l}", [D, 1], F32,
                                           kind="ExternalInput")
    ident_d = nc.dram_tensor("ident16", [128, 128], F16, kind="ExternalInput")
    clsw_d = nc.dram_tensor("clsw", [D, 16], F16, kind="ExternalInput")
    clsb_d = nc.dram_tensor("clsb", [16, 1], F32, kind="ExternalInput")
    out_d = nc.dram_tensor("out_fm", [16, sh], F32, kind="ExternalOutput")

    with tile.TileContext(nc) as tc, ExitStack() as ctx:
        dram = ctx.enter_context(tc.tile_pool(name="dram", bufs=1, space="DRAM"))
        wp = ctx.enter_context(tc.tile_pool(name="weights", bufs=1))
        big = ctx.enter_context(tc.tile_pool(name="big", bufs=1))
        gp = ctx.enter_context(tc.tile_pool(name="gather", bufs=12))
        sp = ctx.enter_context(tc.tile_pool(name="small", bufs=4))
        ck = ctx.enter_context(tc.tile_pool(name="chunk", bufs=3))
        psA = ctx.enter_context(tc.tile_pool(name="psA", bufs=2, space="PSUM"))
        psG = ctx.enter_context(tc.tile_pool(name="psG", bufs=2, space="PSUM"))
        psF = ctx.enter_context(tc.tile_pool(name="psF", bufs=2, space="PSUM"))
        psY = ctx.enter_context(tc.tile_pool(name="psY", bufs=1, space="PSUM"))
        psT = ctx.enter_context(tc.tile_pool(name="psT", bufs=1, space="PSUM"))

        vshard = dram.tile([sh, D], F16, name="vshard")
        vtabH1 = dram.tile([CORES * H1, D], F16, addr_space="Shared",
                           name="vtabH1")
        vtabH2 = dram.tile([CORES * H2, D], F16, addr_space="Shared",
                           name="vtabH2")
        warm_in = dram.tile([D, 2], F32, name="warm_in")
        warm_out = dram.tile([D, 2], F32, addr_space="Shared", name="warm_out")
        bn_in, bn_out = {}, {}
        for l in range(DEPTH):
            for j in (1, 2):
                bn_in[(l, j)] = dram.tile([D, 2], F32, name=f"bni{l}{j}")
                bn_out[(l, j)] = dram.tile([D, 2], F32, addr_space="Shared",
                                           name=f"bno{l}{j}")

        def load(dt_, shape, src, name):
            t = wp.tile(shape, dt_, name=name)
            nc.sync.dma_start(out=t[:], in_=src)
            return t

        idx_sb = [load(I16, [128, nblkR[r] * 8], idx_d[r][:], f"idx_sb{r}")
                  for r in range(2)]
        drel_sb = load(F16, [128, suboff[-1]], drel_d[:], "drel_sb")
        iota_sb = load(F16, [128, maxsub * 128], iota_d[:], "iota_sb")
        invdeg_sb = load(F32, [128, nt], invdeg_d[:], "invdeg_sb")
        mask_sb = load(F16, [1, sh], mask_d[:], "mask_sb")
        wg_sb = [load(F16, [D, D], wg_d[l][:], f"wg_sb{l}") for l in range(DEPTH)]
        w1_sb = [load(F16, [D, H], w1_d[l][:], f"w1_sb{l}") for l in range(DEPTH)]
        fb1_sb = [load(F32, [D, H // D], fb1_d[l][:], f"fb1_sb{l}")
                  for l in range(DEPTH)]
        w2_sb = [[load(F16, [D, D], w2_d[l][h * D:(h + 1) * D, :], f"w2_sb{l}_{h}")
                  for h in range(H // D)] for l in range(DEPTH)]
        bn_sb = {k: load(F32, [D, 1], v[:], f"bn_{k[0]}_{k[1]}")
                 for k, v in bn_d.items()}
        clsw_sb = load(F16, [D, 16], clsw_d[:], "clsw_sb")
        clsb_sb = load(F32, [16, 1], clsb_d[:], "clsb_sb")

        ident16 = load(F16, [128, 128], ident_d[:], "ident16")
        # warm up ncfw collectives during the (gpsimd-idle) L0 phase so the
        # first real AllReduce doesn't pay the first-use cost on the
        # critical path; operates on uninitialized scratch, result unused.
        nc.gpsimd.collective_compute(
            "AllReduce", ALU.add, replica_groups=[list(range(CORES))],
            ins=[warm_in[:]], outs=[warm_out[:]])

        wg1p = wp.tile([D, D], F16, name="wg1p")
        cw2_16 = wp.tile([1, D], F16, name="cw2_16")
        clsw2 = wp.tile([D, 16], F16, name="clsw2")
        clsb2 = wp.tile([16, 1], F32, name="clsb2")

        agg16 = big.tile([D, sh], F16, name="agg16")
        u16 = big.tile([D, sh], F16, name="u16")
        v16 = big.tile([D, sh], F16, name="v16")
        xr16 = big.tile([D, sh], F16, name="xr16")
        nc.sync.dma_start(out=xr16[:], in_=x0_d[:])

        def bn_start(l, j, s2):
            nc.sync.dma_start(out=bn_in[(l, j)][:], in_=s2[:])
            nc.gpsimd.collective_compute(
                "AllReduce", ALU.add, replica_groups=rg,
                ins=[bn_in[(l, j)][:]], outs=[bn_out[(l, j)][:]])
            sums = sp.tile([D, 2], F32, tag="sums", name=f"sums{l}{j}")
            nc.sync.dma_start(out=sums[:], in_=bn_out[(l, j)][:])
            return sums

        def bn_math(l, j, sums, a_out, c_out):
            g_sb = bn_sb[(f"g{j}", l)]
            b_sb = bn_sb[(f"b{j}", l)]
            m = sp.tile([D, 1], F32, tag="bnv", name="m")
            msq = sp.tile([D, 1], F32, tag="bnv", name="msq")
            var = sp.tile([D, 1], F32, tag="bnv", name="var")
            r_ = sp.tile([D, 1], F32, tag="bnv", name="r")
            nc.vector.tensor_scalar_mul(out=m[:], in0=sums[:, 0:1],
                                        scalar1=1.0 / N)
            nc.vector.tensor_scalar_mul(out=msq[:], in0=sums[:, 1:2],
                                        scalar1=1.0 / N)
            nc.vector.tensor_tensor(out=var[:], in0=m[:], in1=m[:], op=ALU.mult)
            nc.vector.tensor_tensor(out=var[:], in0=msq[:], in1=var[:],
                                    op=ALU.subtract)
            nc.vector.tensor_scalar_add(out=var[:], in0=var[:], scalar1=EPS)
            nc.vector.reciprocal(out=r_[:], in_=var[:])
            nc.scalar.activation(out=a_out[:], in_=r_[:], func=AF.Sqrt)
            nc.vector.tensor_tensor(out=a_out[:], in0=g_sb[:], in1=a_out[:],
                                    op=ALU.mult)
            nc.vector.tensor_tensor(out=c_out[:], in0=m[:], in1=a_out[:],
                                    op=ALU.mult)
            nc.vector.tensor_tensor(out=c_out[:], in0=b_sb[:], in1=c_out[:],
                                    op=ALU.subtract)

        def bn_coeffs(l, j, s2, a_out, c_out):
            bn_math(l, j, bn_start(l, j, s2), a_out, c_out)

        nchunks_r = [-(-nblkR[r] // GBLK) for r in range(2)]
        h1_tiles = []
        pA = big.tile([128, nt * 128], F16, name="pA")

        def emit_h1_gather(j):
            nb = min(GBLK, nblkR[0] - GBLK * j)
            gt = gp.tile([128, GBLK * D], F16, tag="G0", name=f"Gh1_{j}")
            h1_tiles.append(gt)
            nidx = nb * BLK
            nc.gpsimd.dma_gather(
                gt[:, :nb * D].rearrange("p (b d) -> p b d", d=D),
                vtabH1[:], idx_sb[0][:, GBLK * 8 * j:GBLK * 8 * j + nb * 8],
                nidx, nidx, D, queue_num=j % 4)

        for l in range(DEPTH):
            # ---- payload: stream (l=0) or batched dma_gather (l=1)
            gtiles = [[], []]
            if l == 1:
                gtiles[0] = h1_tiles
            for r in range((1 if l == 1 else 0), 2):
                for j in range(nchunks_r[r]):
                    nb = min(GBLK, nblkR[r] - GBLK * j)
                    gt = gp.tile([128, GBLK * D], F16, tag=f"G{r}",
                                 name=f"G{l}_{r}_{j}")
                    gtiles[r].append(gt)
                    if l == 0:
                        nc.sync.dma_start(
                            out=gt[:, :nb * D],
                            in_=pay_d[r][:, GBLK * j * D:(GBLK * j + nb) * D])
                    else:
                        view = vtabH1[:] if r == 0 else vtabH2[:]
                        nidx = nb * BLK
                        nc.gpsimd.dma_gather(
                            gt[:, :nb * D].rearrange("p (b d) -> p b d", d=D),
                            view, idx_sb[r][:, GBLK * 8 * j:GBLK * 8 * j + nb * 8],
                            nidx, nidx, D,
                            queue_num=(r * nchunks_r[0] + j) % 4)

            # ---- per-tile segment-matmul aggregation
            # l==0: both ranges in one accumulation. l==1: Phase B only
            # (range-1 blocks); combines with the pA partial built earlier.
            for t in range(nt):
                nbA = int(nblk_t[t][0])
                if l == 0:
                    s0, nbt, rlo = suboff[t], nsub_t[t], 0
                else:
                    s0, nbt, rlo = suboff[t] + nbA, nsub_t[t] - nbA, 1
                St = ck.tile([128, maxsub * 128], F16, tag="S", name=f"S{l}_{t}")
                dr = drel_sb[:, s0:s0 + nbt]
                dr_b = bass.AP(dr.tensor, dr.offset, dr.ap + [[0, 128]])
                nc.vector.tensor_tensor(
                    out=St[:, :nbt * 128].rearrange("p (b j) -> p b j", j=128),
                    in0=iota_sb[:, :nbt * 128].rearrange("p (b j) -> p b j", j=128),
                    in1=dr_b, op=ALU.is_equal)
                ps = psA.tile([128, D], F32, tag="agg", name=f"agg{l}_{t}")
                si = 0
                for r in range(rlo, 2):
                    b0, nb = int(blkoff[t][r]), int(nblk_t[t][r])
                    for bi in range(nb):
                        gb = b0 + bi
                        gt = gtiles[r][gb // GBLK]
                        slot = gb % GBLK
                        nc.tensor.matmul(
                            ps[:], St[:, si * 128:(si + 1) * 128],
                            gt[:, slot * D:(slot + 1) * D],
                            start=(si == 0), stop=(si == nbt - 1))
                        si += 1
                acc2 = sp.tile([128, D], F16, tag="acc2", name=f"acc2{l}_{t}")
                if l == 0:
                    nc.vector.tensor_scalar_mul(out=acc2[:], in0=ps[:],
                                                scalar1=invdeg_sb[:, t:t + 1])
                else:
                    tmp = sp.tile([128, D], F16, tag="tmpB", name=f"tmpB{t}")
                    nc.vector.tensor_scalar_mul(out=tmp[:], in0=ps[:],
                                                scalar1=invdeg_sb[:, t:t + 1])
                    nc.vector.tensor_tensor(
                        out=acc2[:], in0=tmp[:],
                        in1=pA[:, t * 128:(t + 1) * 128], op=ALU.add)
                pv = psT.tile([128, 128], F16, tag="tr", name=f"tr{l}_{t}")
                nc.tensor.transpose(pv[:], acc2[:], ident16[:])
                nc.scalar.activation(out=agg16[:, t * 128:(t + 1) * 128],
                                     in_=pv[:], func=AF.Copy)

            # ---- dense sweep 1: GCN linear + residual -> u; stats of u
            ssum1 = sp.tile([D, nch], F32, tag="ssum", name=f"ssum{l}1")
            ssq1 = sp.tile([D, nch], F32, tag="ssq", name=f"ssq{l}1")
            for ci, (c0, cw) in enumerate(chunks):
                sl = slice(c0, c0 + cw)
                ph = psG.tile([D, CHUNK], F32, tag="gcn", name=f"ph{l}{c0}")
                if l == 0:
                    nc.tensor.matmul(ph[:, :cw], wg_sb[0][:], agg16[:, sl],
                                     start=True, stop=True)
                else:
                    nc.tensor.matmul(ph[:, :cw], wg1p[:], agg16[:, sl],
                                     start=True, stop=False)
                    nc.tensor.matmul(ph[:, :cw], cw2_16[:], mask_sb[:, sl],
                                     start=False, stop=True)
                nc.vector.tensor_tensor(out=u16[:, sl], in0=ph[:, :cw],
                                        in1=xr16[:, sl], op=ALU.add)
                rw = max(0, min(cw, sh_real - c0))
                if rw == 0:
                    nc.vector.memset(ssum1[:, ci:ci + 1], 0.0)
                    nc.vector.memset(ssq1[:, ci:ci + 1], 0.0)
                    continue
                nc.vector.tensor_reduce(out=ssum1[:, ci:ci + 1],
                                        in_=u16[:, c0:c0 + rw],
                                        axis=mybir.AxisListType.X, op=ALU.add)
                sq = ck.tile([D, CHUNK], F16, tag="sq", name=f"sq{l}1{ci}")
                nc.scalar.activation(out=sq[:, :rw], in_=u16[:, c0:c0 + rw],
                                     func=AF.Square,
                                     accum_out=ssq1[:, ci:ci + 1])
            s2a = sp.tile([D, 2], F32, tag="s2", name=f"s2a{l}")
            nc.vector.tensor_reduce(out=s2a[:, 0:1], in_=ssum1[:],
                                    axis=mybir.AxisListType.X, op=ALU.add)
            nc.vector.tensor_reduce(out=s2a[:, 1:2], in_=ssq1[:],
                                    axis=mybir.AxisListType.X, op=ALU.add)
            a1 = sp.tile([D, 1], F32, tag="co", name=f"a1_{l}")
            c1 = sp.tile([D, 1], F32, tag="co", name=f"c1_{l}")
            bn_coeffs(l, 1, s2a, a1, c1)

            # ---- dense sweep 2: BN1 affine -> FF -> v; stats; (l=0) vshard
            ssum2 = sp.tile([D, nch], F32, tag="ssum", name=f"ssum{l}2")
            ssq2 = sp.tile([D, nch], F32, tag="ssq", name=f"ssq{l}2")
            for ci, (c0, cw) in enumerate(chunks):
                sl = slice(c0, c0 + cw)
                xp = ck.tile([D, CHUNK], F16, tag="xp", name=f"xp{l}{c0}")
                nc.vector.tensor_scalar(out=xp[:, :cw], in0=u16[:, sl],
                                        scalar1=a1[:], scalar2=c1[:],
                                        op0=ALU.mult, op1=ALU.add)
                py = psY.tile([D, CHUNK], F32, tag="ff2", name=f"py{l}{c0}")
                for h in range(H // D):
                    pr = psF.tile([D, CHUNK], F32, tag="ff1",
                                  name=f"pr{l}{c0}{h}")
                    nc.tensor.matmul(pr[:, :cw], w1_sb[l][:, h * D:(h + 1) * D],
                                     xp[:, :cw], start=True, stop=True)
                    rh = ck.tile([D, CHUNK], F16, tag="rh", name=f"rh{l}{c0}{h}")
                    nc.scalar.activation(out=rh[:, :cw], in_=pr[:, :cw],
                                         func=AF.Relu, bias=fb1_sb[l][:, h:h + 1],
                                         scale=1.0)
                    nc.tensor.matmul(py[:, :cw], w2_sb[l][h][:], rh[:, :cw],
                                     start=(h == 0), stop=(h == H // D - 1))
                nc.vector.tensor_tensor(out=v16[:, sl], in0=py[:, :cw],
                                        in1=xp[:, :cw], op=ALU.add)
                rw = max(0, min(cw, sh_real - c0))
                if l == 0 and rw < cw:
                    nc.vector.memset(v16[:, c0 + rw:c0 + cw], 0.0)
                if rw > 0:
                    nc.vector.tensor_reduce(out=ssum2[:, ci:ci + 1],
                                            in_=v16[:, c0:c0 + rw],
                                            axis=mybir.AxisListType.X,
                                            op=ALU.add)
                    sq = ck.tile([D, CHUNK], F16, tag="sq", name=f"sq{l}2{ci}")
                    nc.scalar.activation(out=sq[:, :rw], in_=v16[:, c0:c0 + rw],
                                         func=AF.Square,
                                         accum_out=ssq2[:, ci:ci + 1])
                else:
                    nc.vector.memset(ssum2[:, ci:ci + 1], 0.0)
                    nc.vector.memset(ssq2[:, ci:ci + 1], 0.0)
                if l == 0:
                    for t in range(c0 // 128, (c0 + cw) // 128):
                        pv = psT.tile([128, 128], F16, tag="tr", name=f"tv{t}")
                        nc.tensor.transpose(pv[:], v16[:, t * 128:(t + 1) * 128],
                                            ident16[:])
                        vT = sp.tile([128, D], F16, tag="vT", name=f"vT{t}")
                        nc.scalar.activation(out=vT[:], in_=pv[:], func=AF.Copy)
                        nc.sync.dma_start(out=vshard[t * 128:(t + 1) * 128, :],
                                          in_=vT[:])
                    if c0 + cw == H1:
                        nc.gpsimd.collective_compute(
                            "AllGather", ALU.bypass, replica_groups=rg,
                            ins=[vshard[0:H1, :]], outs=[vtabH1[:]])
            s2b = sp.tile([D, 2], F32, tag="s2", name=f"s2b{l}")
            nc.vector.tensor_reduce(out=s2b[:, 0:1], in_=ssum2[:],
                                    axis=mybir.AxisListType.X, op=ALU.add)
            nc.vector.tensor_reduce(out=s2b[:, 1:2], in_=ssq2[:],
                                    axis=mybir.AxisListType.X, op=ALU.add)

            if l == 0:
                # H1 gathers start as soon as AG1 lands; AG2 + AR2 triggers
                # are interleaved so their doorbells fire on time.
                for j in range(min(48, nchunks_r[0])):
                    emit_h1_gather(j)
                sums02 = bn_start(l, 2, s2b)
                nc.gpsimd.collective_compute(
                    "AllGather", ALU.bypass, replica_groups=rg,
                    ins=[vshard[H1:sh, :]], outs=[vtabH2[:]])
                for j in range(min(48, nchunks_r[0]), nchunks_r[0]):
                    emit_h1_gather(j)
                # Phase A: range-0 blocks -> invdeg-scaled fp16 partial in pA
                for t in range(nt):
                    nbA = int(nblk_t[t][0])
                    s0 = suboff[t]
                    StA = ck.tile([128, maxsub * 128], F16, tag="S",
                                  name=f"SA_{t}")
                    dr = drel_sb[:, s0:s0 + nbA]
                    dr_b = bass.AP(dr.tensor, dr.offset, dr.ap + [[0, 128]])
                    nc.vector.tensor_tensor(
                        out=StA[:, :nbA * 128].rearrange(
                            "p (b j) -> p b j", j=128),
                        in0=iota_sb[:, :nbA * 128].rearrange(
                            "p (b j) -> p b j", j=128),
                        in1=dr_b, op=ALU.is_equal)
                    psa = psA.tile([128, D], F32, tag="agg", name=f"aggA_{t}")
                    b0 = int(blkoff[t][0])
                    for bi in range(nbA):
                        gb = b0 + bi
                        gt = h1_tiles[gb // GBLK]
                        slot = gb % GBLK
                        nc.tensor.matmul(
                            psa[:], StA[:, bi * 128:(bi + 1) * 128],
                            gt[:, slot * D:(slot + 1) * D],
                            start=(bi == 0), stop=(bi == nbA - 1))
                    nc.scalar.activation(out=pA[:, t * 128:(t + 1) * 128],
                                         in_=psa[:], func=AF.Copy,
                                         scale=invdeg_sb[:, t:t + 1])
                a2 = sp.tile([D, 1], F32, tag="co", name="a2_0")
                c2 = sp.tile([D, 1], F32, tag="co", name="c2_0")
                bn_math(l, 2, sums02, a2, c2)
                nc.vector.tensor_scalar_mul(out=wg1p[:], in0=wg_sb[1][:],
                                            scalar1=a2[:])
                c2_16 = sp.tile([D, 1], F16, tag="c216", name="c2_16")
                nc.vector.tensor_copy(out=c2_16[:], in_=c2[:])
                pcw = psG.tile([D, CHUNK], F32, tag="gcn", name="pcw2")
                nc.tensor.matmul(pcw[0:1, 0:D], c2_16[:], wg_sb[1][:],
                                 start=True, stop=True)
                nc.scalar.activation(out=cw2_16[:], in_=pcw[0:1, 0:D],
                                     func=AF.Copy)
                nc.vector.tensor_scalar(out=xr16[:], in0=v16[:],
                                        scalar1=a2[:], scalar2=c2[:],
                                        op0=ALU.mult, op1=ALU.add)
            else:
                a2p = sp.tile([D, 1], F32, tag="co", name="a2_1")
                c2p = sp.tile([D, 1], F32, tag="co", name="c2_1")
                bn_coeffs(l, 2, s2b, a2p, c2p)
                nc.vector.tensor_scalar_mul(out=clsw2[:], in0=clsw_sb[:],
                                            scalar1=a2p[:])
                c2p_16 = sp.tile([D, 1], F16, tag="c216", name="c2p_16")
                nc.vector.tensor_copy(out=c2p_16[:], in_=c2p[:])
                pcb = psY.tile([D, CHUNK], F32, tag="ff2", name="pcb")
                nc.tensor.matmul(pcb[0:16, 0:1], clsw_sb[:], c2p_16[:],
                                 start=True, stop=True)
                nc.vector.tensor_tensor(out=clsb2[:], in0=pcb[0:16, 0:1],
                                        in1=clsb_sb[:], op=ALU.add)
                for c0, cw in chunks:
                    sl = slice(c0, c0 + cw)
                    pc = psY.tile([D, CHUNK], F32, tag="ff2", name=f"pc{c0}")
                    nc.tensor.matmul(pc[0:16, :cw], clsw2[:], v16[:, sl],
                                     start=True, stop=True)
                    oc = ck.tile([16, CHUNK], F32, tag="oc", name=f"oc{c0}")
                    nc.scalar.activation(out=oc[:, :cw], in_=pc[0:16, :cw],
                                         func=AF.Identity, bias=clsb2[:],
                                         scale=1.0)
                    nc.sync.dma_start(out=out_d[:, sl], in_=oc[:, :cw])

    nc.compile()
    return nc


# ----------------------------------------------------------------------------
# Entry points
# ----------------------------------------------------------------------------

def _make_in_maps(cfg, inputs):
    W_gcn = np.asarray(inputs["W_gcn"], np.float32)
    ff_w1 = np.asarray(inputs["ff_w1"], np.float32)
    ff_b1 = np.asarray(inputs["ff_b1"], np.float32)
    ff_w2 = np.asarray(inputs["ff_w2"], np.float32)
    cls_w = np.asarray(inputs["cls_w"], np.float32)
    cls_b = np.asarray(inputs["cls_b"], np.float32)

    shared = {
        "clsw": np.ascontiguousarray(cls_w.astype(np.float16)),
        "clsb": np.ascontiguousarray(cls_b.reshape(16, 1)),
        "iotar": cfg["iota_rep"],
        "ident16": np.ascontiguousarray(np.eye(128, dtype=np.float16)),
    }
    for l in range(DEPTH):
        shared[f"wg{l}"] = np.ascontiguousarray(W_gcn[l].astype(np.float16))
        shared[f"w1_{l}"] = np.ascontiguousarray(ff_w1[l].astype(np.float16))
        shared[f"fb1_{l}"] = np.ascontiguousarray(
            ff_b1[l].reshape(H // D, D).T)
        shared[f"w2_{l}"] = np.ascontiguousarray(ff_w2[l].astype(np.float16))
        shared[f"g1_{l}"] = np.ascontiguousarray(
            np.asarray(inputs["bn1_g"], np.float32)[l].reshape(D, 1))
        shared[f"b1_{l}"] = np.ascontiguousarray(
            np.asarray(inputs["bn1_b"], np.float32)[l].reshape(D, 1))
        shared[f"g2_{l}"] = np.ascontiguousarray(
            np.asarray(inputs["bn2_g"], np.float32)[l].reshape(D, 1))
        shared[f"b2_{l}"] = np.ascontiguousarray(
            np.asarray(inputs["bn2_b"], np.float32)[l].reshape(D, 1))

    sh = cfg["sh"]
    in_maps = []
    for c in range(CORES):
        m = dict(shared)
        m["x016"] = np.ascontiguousarray(
            cfg["table0"][c * sh:(c + 1) * sh].T.astype(np.float16))
        m["pay0"] = cfg["pay"][0][c]
        m["pay1"] = cfg["pay"][1][c]
        m["idx0"] = cfg["idxw"][0][c]
        m["idx1"] = cfg["idxw"][1][c]
        m["drel"] = np.ascontiguousarray(cfg["drel"][c])
        m["invdeg"] = np.ascontiguousarray(cfg["invdeg"][c])
        m["mask16"] = np.ascontiguousarray(cfg["mask"][c].reshape(1, sh))
        in_maps.append(m)
    return in_maps


def _postprocess(cfg, results):
    sh, sh_real = cfg["sh"], cfg["sh_real"]
    N = cfg["N"]
    node_of_tok = cfg["node_of_tok"]
    out = np.empty((N, 16), np.float32)
    for c in range(CORES):
        arr = results[c]["out_fm"]
        toks = np.arange(c * sh, c * sh + sh_real)
        out[node_of_tok[toks]] = arr.T[:sh_real]
    return out


def _ensure_axon_hooks():
    try:
        import antenv.axon_hooks  # noqa: F401
        return
    except ImportError:
        pass
    import types
    import antenv
    mod = types.ModuleType("antenv.axon_hooks")
    mod._hook = None

    def set_axon_ntff_profile_hook(h):
        mod._hook = h

    def get_axon_ntff_profile_hook():
        return mod._hook

    mod.set_axon_ntff_profile_hook = set_axon_ntff_profile_hook
    mod.get_axon_ntff_profile_hook = get_axon_ntff_profile_hook
    sys.modules["antenv.axon_hooks"] = mod
    antenv.axon_hooks = mod
    try:
        from trn_agent_boot.trn_boot import _ntff_profile_via_ctypes
        h = _ntff_profile_via_ctypes("/opt/axon/libaxon_pjrt.so")
        if h is not None:
            mod._hook = h
    except Exception as e:  # pragma: no cover
        print(f"ntff hook setup failed: {e}", file=sys.stderr)


_CACHE = {}


def run(trace=False, **inputs):
    if trace:
        _ensure_axon_hooks()
    nodes = np.asarray(inputs["nodes"], np.float32)
    edge_src = np.asarray(inputs["edge_src"], np.int64)
    edge_dst = np.asarray(inputs["edge_dst"], np.int64)
    cfg = _prepare(nodes, edge_src, edge_dst)

    key = (nodes.shape, len(edge_src), int(cfg["suboff"][-1]))
    if key not in _CACHE:
        _CACHE[key] = build_program(cfg)
    nc = _CACHE[key]

    in_maps = _make_in_maps(cfg, inputs)
    res = run_bass_kernel_spmd(nc, in_maps, list(range(CORES)), trace=trace)
    return _postprocess(cfg, res.results), res


def kernel(**inputs) -> np.ndarray:
    out, _ = run(trace=False, **inputs)
    return out
